# revision 43
# baseline (speedup 1.0000x reference)
"""Trainium2 Bass kernel for nn_GRU_15461882266204 (minGRU with causal conv gate).

Math (reference):
  w0 = x @ w_w.T ; z0 = x @ wz_w.T ; th = x @ wh_w.T          (S,H)
  z  = sigmoid(causal_conv4(z0, conv_w, segment-masked))
  a  = (1-z) * (1-start) ; b = z * th
  h_t = a_t * h_{t-1} + b_t                                    (scan over S)
  out = (h * silu(w0)) @ wo_w.T                                (S,D)

Strategy: sequence-parallel over 8 NeuronCores (1024 positions each, all 5632
channels per core). Phases:
  A1: per hidden m-tile: z/ht projections (bf16 or fp8-DoubleRow for z),
      causal conv + gates on DVE with host-precomputed boundary masks and
      host-precomputed 3-column z_pre history (kills all narrow matmuls),
      hardware tensor_tensor_scan for h_loc and the cumprod A. h_loc/A go to
      DRAM in bf16 (one merged DMA); chunk-end summaries stay in SBUF.
  B:  360KB AllGather of (A_end, h_end) summaries; every core redundantly
      computes the carry chain. Runs concurrently with A2.
  A2: w0 projection + silu, kept resident in SBUF (bf16).
  D:  fused carry fixup + down-projection: per output block, per m-tile:
      g = (h_loc + A*carry) * silu on DVE feeding bf16 matmuls accumulating
      over all 44 m-tiles in PSUM. Output is sequence-sharded; host concats.
"""
import sys

sys.path.insert(0, "/opt/trn_rl_repo")

import numpy as np

import concourse.bacc as bacc
import concourse.mybir as mybir
import concourse.tile as tile
from concourse.bass_utils import run_bass_kernel_spmd

import ml_dtypes

BF16 = np.dtype(ml_dtypes.bfloat16)
FP8 = np.dtype(ml_dtypes.float8_e4m3)

F32 = mybir.dt.float32
MBF16 = mybir.dt.bfloat16
MFP8 = mybir.dt.float8e4
AL = mybir.AluOpType
ACTF = mybir.ActivationFunctionType
PERF_DR = mybir.MatmulPerfMode.DoubleRow

P = 128
CONV = 4
# fp8 (e4m3, DoubleRow) for the z projection: its error is damped by the
# sigmoid; x scaled by 16, wz by 64 on host, descaled at PSUM drain.
FP8_Z = True
XSCALE = 16.0
WSCALE = 64.0
DESCALE = 1.0 / (XSCALE * WSCALE)


def build_gru_kernel(D, H, SC, NC, fp8_z=FP8_Z):
    KT = D // P          # contraction k-tiles (16)
    K2 = KT // 2         # fp8 DoubleRow k-tiles (8)
    MT = H // P          # hidden m-tiles (44)
    SCH = SC + 3         # z_pre cols incl 3 history cols
    MPT = SC // P        # seq row-tiles (8)

    nc = bacc.Bacc(None, target_bir_lowering=False, debug=False)

    xt_in = nc.declare_dram_parameter("xt", [P, KT, SC], MBF16, isOutput=False)
    wz_in = (
        nc.declare_dram_parameter("wz8", [MT, P, K2, 2, P], MFP8, isOutput=False)
        if fp8_z
        else nc.declare_dram_parameter("wz", [MT, P, KT, P], MBF16, isOutput=False)
    )
    if fp8_z:
        xt8_in = nc.declare_dram_parameter(
            "xt8", [P, K2, 2, SC], MFP8, isOutput=False
        )
    wh_in = nc.declare_dram_parameter("wh", [MT, P, KT, P], MBF16, isOutput=False)
    w_in = nc.declare_dram_parameter("w", [MT, P, KT, P], MBF16, isOutput=False)
    wo_in = nc.declare_dram_parameter("wo", [MT, P, D], MBF16, isOutput=False)
    # czh[:,m]: cols 0-3 conv_w taps, 4-6 host z_pre history, 7 pad
    czh_in = nc.declare_dram_parameter("czh", [P, MT, 8], F32, isOutput=False)
    # padded to 1056 cols: 64B-aligned rows (odd-size DMAs are slow)
    u_in = nc.declare_dram_parameter("u", [P, SC + 32], MBF16, isOutput=False)
    sel_in = nc.declare_dram_parameter("sel", [P, NC], F32, isOutput=False)
    out_d = nc.declare_dram_parameter("out", [SC, D], F32, isOutput=True)

    with tile.TileContext(nc) as tc:
        with (
            tc.tile_pool(name="const", bufs=1) as cpool,
            tc.tile_pool(name="wts", bufs=2) as wpool,
            tc.tile_pool(name="work", bufs=2) as wk,
            tc.tile_pool(name="dload", bufs=4) as dl,
            tc.tile_pool(name="psum", bufs=8, space="PSUM") as pp,
            tc.tile_pool(name="dram", bufs=1, space="DRAM") as dp,
        ):
            # ---- resident tiles ------------------------------------------------
            # xt on the sync queue; u/sel on the scalar (Activation) HWDGE
            # queue so the first m-tile's weight DMAs start right behind xt.
            if fp8_z:
                # z runs first per m-tile: its fp8 activations lead the queue
                xt8_sb = cpool.tile([P, K2, 2, SC], MFP8, tag="xt8")
                nc.scalar.dma_start(xt8_sb[:], xt8_in[:])
            xt_sb = cpool.tile([P, KT, SC], MBF16, tag="xt")
            nc.sync.dma_start(xt_sb[:, :, 0:512], xt_in[:, :, 0:512])
            nc.scalar.dma_start(xt_sb[:, :, 512:SC], xt_in[:, :, 512:SC])
            u_sb = cpool.tile([P, SC + 32], MBF16, tag="u")
            nc.scalar.dma_start(u_sb[:], u_in[:])
            sel_sb = cpool.tile([P, NC], F32, tag="sel")
            nc.scalar.dma_start(sel_sb[:], sel_in[:])
            # all 44 m-tiles' conv taps + z history in ONE dma (tiny rows are
            # descriptor-bound: 44 separate [P,8] DMAs cost ~8us each)
            czh_sb = cpool.tile([P, MT, 8], F32, tag="czh")
            nc.scalar.dma_start(czh_sb[:], czh_in[:])
            summA = cpool.tile([P, 64], F32, tag="summA")
            summH = cpool.tile([P, 64], F32, tag="summH")
            silu_sb = cpool.tile([P, MT, SC], MBF16, tag="silu")

            # internal DRAM bounce buffers
            hlA_d = dp.tile([MT, P, 2, SC], MBF16)  # [:,0,:]=A  [:,1,:]=h_loc
            summ_d = dp.tile([P, 128], F32)
            gath_d = dp.tile([NC, P, 128], F32, addr_space="Shared")

            # ---- phase A1: z/ht matmuls, conv, gating, local scans -------------
            scopeA = nc.named_scope("phaseA1"); scopeA.__enter__()
            for m in range(MT):
                czh = czh_sb[:, m]
                if fp8_z:
                    wz_sb = wpool.tile([P, K2, 2, P], MFP8, tag="wz8")
                else:
                    wz_sb = wpool.tile([P, KT, P], MBF16, tag="wz")
                nc.sync.dma_start(wz_sb[:], wz_in[m])
                wh_sb = wpool.tile([P, KT, P], MBF16, tag="wh")
                nc.sync.dma_start(wh_sb[:], wh_in[m])

                # z_pre: cols 0-2 = host history, cols 3.. = matmul
                zp = wk.tile([P, SCH], MBF16 if fp8_z else F32, tag="zpre")
                nc.scalar.copy(zp[:, 0:3], czh[:, 4:7])
                for half in (0, 1):
                    ps = pp.tile([P, 512], F32, tag="ps")
                    if fp8_z:
                        for k2 in range(K2):
                            nc.tensor.matmul(
                                ps[:],
                                wz_sb[:, k2],
                                xt8_sb[:, k2, :, half * 512 : half * 512 + 512],
                                start=(k2 == 0),
                                stop=(k2 == K2 - 1),
                                perf_mode=PERF_DR,
                            )
                        nc.scalar.activation(
                            zp[:, 3 + half * 512 : 3 + half * 512 + 512],
                            ps[:],
                            ACTF.Copy,
                            scale=DESCALE,
                        )
                    else:
                        for k in range(KT):
                            nc.tensor.matmul(
                                ps[:],
                                wz_sb[:, k],
                                xt_sb[:, k, half * 512 : half * 512 + 512],
                                start=(k == 0),
                                stop=(k == KT - 1),
                            )
                        nc.scalar.copy(
                            zp[:, 3 + half * 512 : 3 + half * 512 + 512], ps[:]
                        )

                ps_h = []
                for half in (0, 1):
                    ps = pp.tile([P, 512], F32, tag="ps")
                    for k in range(KT):
                        nc.tensor.matmul(
                            ps[:],
                            wh_sb[:, k],
                            xt_sb[:, k, half * 512 : half * 512 + 512],
                            start=(k == 0),
                            stop=(k == KT - 1),
                        )
                    ps_h.append(ps)

                # conv: yk(t) = u(t)*y{k-1}(t-1), folded in-place into one tile
                y = wk.tile([P, SC + 2], MBF16 if fp8_z else F32, tag="y")
                nc.vector.tensor_tensor(
                    y[:], u_sb[:, : SC + 2], zp[:, : SC + 2], AL.mult
                )
                acc = wk.tile([P, SC], F32, tag="acc")
                nc.vector.tensor_scalar(
                    acc[:], zp[:, 3:SCH], czh[:, 3:4], None, AL.mult
                )
                nc.vector.scalar_tensor_tensor(
                    acc[:], y[:, 2 : SC + 2], czh[:, 2:3], acc[:], AL.mult, AL.add
                )
                nc.vector.tensor_tensor(
                    y[:, : SC + 1], u_sb[:, 1 : SC + 2], y[:, : SC + 1], AL.mult
                )
                nc.vector.scalar_tensor_tensor(
                    acc[:], y[:, 1 : SC + 1], czh[:, 1:2], acc[:], AL.mult, AL.add
                )
                nc.vector.tensor_tensor(
                    y[:, :SC], u_sb[:, 2 : SC + 2], y[:, :SC], AL.mult
                )
                nc.vector.scalar_tensor_tensor(
                    acc[:], y[:, :SC], czh[:, 0:1], acc[:], AL.mult, AL.add
                )

                zt = wk.tile([P, SC], F32, tag="zt")
                nc.scalar.activation(zt[:], acc[:], ACTF.Sigmoid)
                na = wk.tile([P, SC + 2], MBF16 if fp8_z else F32, tag="y", name="na")
                nc.scalar.activation(na[:, :SC], acc[:], ACTF.Sigmoid, scale=-1.0)
                # a = (1-z)*u, in place
                nc.vector.tensor_tensor(
                    na[:, :SC], na[:, :SC], u_sb[:, 2 : SC + 2], AL.mult
                )
                # b = z*th, in place over zt
                for half, ps in zip((0, 1), ps_h):
                    nc.vector.tensor_tensor(
                        zt[:, half * 512 : half * 512 + 512],
                        zt[:, half * 512 : half * 512 + 512],
                        ps[:],
                        AL.mult,
                    )

                hlA = wk.tile([P, 2, SC], MBF16, tag="hlA")
                nc.vector.tensor_tensor_scan(
                    hlA[:, 1, :], na[:, :SC], zt[:], 0.0, AL.mult, AL.add
                )
                # A-scan: u==1 wherever a!=0, so a*u*A == a*A
                nc.vector.tensor_tensor_scan(
                    hlA[:, 0, :], na[:, :SC], u_sb[:, 2 : SC + 2], 1.0,
                    AL.mult, AL.mult,
                )
                nc.scalar.copy(summA[:, m : m + 1], hlA[:, 0, SC - 1 : SC])
                nc.scalar.copy(summH[:, m : m + 1], hlA[:, 1, SC - 1 : SC])
                nc.sync.dma_start(hlA_d[m], hlA[:])
            scopeA.__exit__(None, None, None)

            # ---- phase B (launch): AllGather of scan summaries -----------------
            # Only the summary DMAs + the collective itself go here (gpsimd
            # queue) so nothing downstream head-blocks the sync queue while
            # the collective is in flight; the gather readback + carry chain
            # are emitted after phase A2.
            nc.scalar.dma_start(summ_d[:, 0:64], summA[:])
            nc.scalar.dma_start(summ_d[:, 64:128], summH[:])
            nc.gpsimd.collective_compute(
                "AllGather",
                AL.bypass,
                replica_groups=[list(range(NC))],
                ins=[summ_d.opt()],
                outs=[gath_d.opt()],
            )

            # ---- phase A2: w0 projection + silu, resident in SBUF --------------
            # The carry-chain consumption (gather readback + 16 tiny DVE ops)
            # is emitted mid-loop, once the collective is certainly complete,
            # so phase D starts unblocked the moment A2's matmuls finish.
            state = cpool.tile([P, MT], F32, tag="cstate")
            tmp_c = cpool.tile([P, MT], F32, tag="ctmp")
            mycarry = cpool.tile([P, MT], F32, tag="mycarry")
            scopeC = nc.named_scope("phaseA2"); scopeC.__enter__()
            for m in range(MT):
                w_sb = wpool.tile([P, KT, P], MBF16, tag="wh", name="w_sb")
                nc.sync.dma_start(w_sb[:], w_in[m])
                for half in (0, 1):
                    ps = pp.tile([P, 512], F32, tag="ps")
                    for k in range(KT):
                        nc.tensor.matmul(
                            ps[:],
                            w_sb[:, k],
                            xt_sb[:, k, half * 512 : half * 512 + 512],
                            start=(k == 0),
                            stop=(k == KT - 1),
                        )
                    nc.scalar.activation(
                        silu_sb[:, m, half * 512 : half * 512 + 512],
                        ps[:],
                        ACTF.Silu,
                    )
                if m == 11:
                    # ---- phase B (consume): redundant carry chain --------------
                    # gather readback issues from the idle gpsimd queue: its
                    # wait on the collective must not head-block the sync
                    # queue that feeds A2's weight stream.
                    nc.vector.memset(state[:], 0.0)
                    nc.vector.memset(mycarry[:], 0.0)
                    for r in range(NC):
                        gs = wk.tile([P, SC], F32, tag="zt", name=f"gs{r}")
                        nc.gpsimd.dma_start(gs[:, 0:128], gath_d[r])
                        if r > 0:
                            nc.vector.scalar_tensor_tensor(
                                mycarry[:], state[:], sel_sb[:, r : r + 1],
                                mycarry[:], AL.mult, AL.add,
                            )
                        if r < NC - 1:
                            nc.vector.tensor_tensor(
                                tmp_c[:], state[:], gs[:, 0:MT], AL.mult
                            )
                            nc.vector.tensor_tensor(
                                state[:], tmp_c[:], gs[:, 64 : 64 + MT], AL.add
                            )
            scopeC.__exit__(None, None, None)

            # ---- phase D: fused carry fixup + down-projection ------------------
            # Flat (block, m) step list with DMA lookahead: loads for the next
            # block are emitted before the previous block's PSUM drain, so
            # drain waits never head-block the load queues (hlA on sync, wo on
            # scalar).
            scopeD = nc.named_scope("phaseD"); scopeD.__enter__()
            steps = [
                (mb, nbb, m) for mb in (0, 1) for nbb in (0, 1) for m in range(MT)
            ]
            LOOK = 3
            loaded = {}

            def issue_load(step):
                mb, nbb, m = step
                hlA_rd = dl.tile([P, 2, 512], MBF16, tag="hlard")
                nc.sync.dma_start(
                    hlA_rd[:], hlA_d[m][:, :, mb * 512 : mb * 512 + 512]
                )
                wo_rd = dl.tile([P, 1024], MBF16, tag="word")
                nc.scalar.dma_start(
                    wo_rd[:], wo_in[m][:, nbb * 1024 : nbb * 1024 + 1024]
                )
                loaded[step] = (hlA_rd, wo_rd)

            for j0 in range(LOOK):
                issue_load(steps[j0])
            ps_o = None
            for i, step in enumerate(steps):
                if i + LOOK < len(steps):
                    issue_load(steps[i + LOOK])
                mb, nbb, m = step
                hlA_rd, wo_rd = loaded.pop(step)
                if m == 0:
                    ps_o = [
                        pp.tile([P, 512], F32, tag="ps", name=f"pso{mb}{nbb}{k}")
                        for k in range(8)
                    ]
                t1 = wk.tile([P, 512], MBF16, tag="g", name="t1")
                nc.vector.scalar_tensor_tensor(
                    t1[:], hlA_rd[:, 0, :], mycarry[:, m : m + 1],
                    hlA_rd[:, 1, :], AL.mult, AL.add,
                )
                g = wk.tile([P, 512], MBF16, tag="g")
                nc.vector.tensor_tensor(
                    g[:], t1[:], silu_sb[:, m, mb * 512 : mb * 512 + 512], AL.mult
                )
                for i_m in range(4):
                    for j in range(2):
                        nc.tensor.matmul(
                            ps_o[i_m * 2 + j][:],
                            g[:, i_m * P : (i_m + 1) * P],
                            wo_rd[:, j * 512 : j * 512 + 512],
                            start=(m == 0),
                            stop=(m == MT - 1),
                        )
                if m == MT - 1:
                    # drain on scalar+vector; out DMAs from the scalar queue
                    for i_m in range(4):
                        for j in range(2):
                            o_sb = wk.tile([P, 512], F32, tag="osb")
                            if j == 0:
                                nc.scalar.copy(o_sb[:], ps_o[i_m * 2 + j][:])
                            else:
                                nc.vector.tensor_copy(o_sb[:], ps_o[i_m * 2 + j][:])
                            nc.scalar.dma_start(
                                out_d[
                                    (mb * 4 + i_m) * P : (mb * 4 + i_m + 1) * P,
                                    nbb * 1024 + j * 512 : nbb * 1024
                                    + j * 512
                                    + 512,
                                ],
                                o_sb[:],
                            )
            scopeD.__exit__(None, None, None)
    nc.compile()
    return nc


def _prep_inputs(x, cu_seqlens, w_w, wz_w, wh_w, wo_w, conv_w, NC, fp8_z=FP8_Z):
    """Host-side sharding + layout prep. Returns in_maps list."""
    S, D = x.shape[1], x.shape[2]
    H = w_w.shape[0]
    SC = S // NC
    KT, MT = D // P, H // P
    K2 = KT // 2

    xT = np.ascontiguousarray(x[0].T.astype(np.float32))  # (D, S)

    start = np.zeros(S, np.float32)
    for v in np.asarray(cu_seqlens[:-1]):
        v = int(v)
        if 0 <= v < S:
            start[v] = 1.0
    u = 1.0 - start
    u_full = np.ones(S + 32, np.float32)
    u_full[2 : S + 2] = u  # index j <-> position j-2

    def wprep(wm):  # (H, D) -> (MT, P, KT, P) with [m,p,k,j] = w[m*P+j, k*P+p]
        return np.ascontiguousarray(
            wm.astype(np.float32).reshape(MT, P, KT, P).transpose(0, 3, 2, 1)
        ).astype(BF16)

    wz_f = np.asarray(wz_w, np.float32)
    wh_t, w_t = wprep(wh_w), wprep(w_w)
    if fp8_z:
        wz64 = np.clip(wz_f * WSCALE, -240, 240).astype(FP8)
        wz_t = np.ascontiguousarray(
            wz64.reshape(MT, P, K2, 2, P).transpose(0, 4, 2, 3, 1)
        )
        x16 = np.clip(xT * XSCALE, -240, 240).astype(FP8)
    else:
        wz_t = wprep(wz_w)
    wo_t = np.ascontiguousarray(
        wo_w.T.astype(np.float32).reshape(MT, P, D)
    ).astype(BF16)

    cw_t = conv_w.astype(np.float32)  # (H, CONV)

    in_maps = []
    for c in range(NC):
        s0 = c * SC
        xt_c = np.ascontiguousarray(
            xT[:, s0 : s0 + SC].reshape(KT, P, SC).transpose(1, 0, 2)
        ).astype(BF16)
        # host z_pre history: 3 cols before s0 (zeros at t<0)
        xh = np.zeros((D, 3), np.float32)
        lo = max(0, s0 - 3)
        if s0 > 0:
            xh[:, 3 - (s0 - lo) :] = xT[:, lo:s0]
        zh = wz_f @ xh  # (H, 3)
        czh_c = np.zeros((MT, P, 8), np.float32)
        czh_c[:, :, 0:CONV] = cw_t.reshape(MT, P, CONV)
        czh_c[:, :, 4:7] = zh.reshape(MT, P, 3)
        czh_c = np.ascontiguousarray(czh_c.transpose(1, 0, 2))  # (P, MT, 8)
        u_c = np.ascontiguousarray(
            np.broadcast_to(u_full[s0 : s0 + SC + 32], (P, SC + 32))
        ).astype(BF16)
        sel_c = np.zeros((NC,), np.float32)
        sel_c[c] = 1.0
        sel_c = np.ascontiguousarray(np.broadcast_to(sel_c, (P, NC)))
        imap = {
            "xt": xt_c,
            "wh": wh_t,
            "w": w_t,
            "wo": wo_t,
            "czh": czh_c,
            "u": u_c,
            "sel": sel_c,
        }
        if fp8_z:
            imap["wz8"] = wz_t
            imap["xt8"] = np.ascontiguousarray(
                x16[:, s0 : s0 + SC].reshape(K2, 2, P, SC).transpose(2, 0, 1, 3)
            )
        else:
            imap["wz"] = wz_t
        in_maps.append(imap)
    return in_maps


_NC_CACHE = {}


def run_gru(x, cu_seqlens, w_w, wz_w, wh_w, wo_w, conv_w, NC=8, trace=False):
    S, D = x.shape[1], x.shape[2]
    H = w_w.shape[0]
    SC = S // NC
    key = (D, H, SC, NC, FP8_Z)
    if key not in _NC_CACHE:
        _NC_CACHE[key] = build_gru_kernel(D, H, SC, NC)
    nc = _NC_CACHE[key]
    in_maps = _prep_inputs(x, cu_seqlens, w_w, wz_w, wh_w, wo_w, conv_w, NC)
    res = run_bass_kernel_spmd(nc, in_maps, list(range(NC)), trace=trace)
    out = np.concatenate([res.results[c]["out"] for c in range(NC)], axis=0)
    return out.reshape(1, S, D).astype(np.float32), res


def kernel(**inputs):
    out, _ = run_gru(
        inputs["x"],
        inputs["cu_seqlens"],
        inputs["w_w"],
        inputs["wz_w"],
        inputs["wh_w"],
        inputs["wo_w"],
        inputs["conv_w"],
        NC=8,
    )
    return out
